# revision 25
# baseline (speedup 1.0000x reference)
"""Trainium2 Bass kernel for nn_Dictionary (vq_codebook): out = inp @ Q.T, Q from QR(weight+1e-8).

Strategy (per sharding_hint): data-parallel over batch B=131072 across 8 cores
(16384 rows each); Q.T replicated on every core (QR is tiny, computed on host).

Default mode "f16t2" (_build_t2): host transposes inp so the contraction dim i
lands on SBUF partitions with contiguous DMAs, operands in fp16 (1 cyc/row on
the PE, fp32 PSUM accumulation -> rel L2 err ~3.6e-4). Per core: stationary =
128x128 blocks of Q.T, moving = [128i, 512b] slices of inpT chunks, PSUM holds
out.T [128j, 512b] accumulated over 4 i-tiles (it-outer order so matmuls start
as each i-tile lands); DVE/ACT cast-copy PSUM into [128, <=4096] fp16 out.T
group tiles; host transposes back and upcasts to fp32.

The steady-state PE stream is at the fp16 floor (512 matmuls x 215.8 ns =
110.5us; median inter-MM gap = 216 ns), so v2 targets the edges (132.4 ->
~128.5us median):
  - 26 warmup matmuls on a DVE-memset zero tile keep the PE busy from ~7.5us
    (right after the fixed ~7us Tile/NEFF preamble) so the HAM clock gate
    un-throttles to 2.4 GHz around when real data lands (~10.2us)
  - qT rides the sync ring as 4 per-i-tile DMAs (lands ~9.3us); input chunks
    ride the scalar ring with a small leading chunk (512 cols)
  - input prefetch: 10 chunks, inpool bufs=5; output groups [3x4096, 2048,
    1024, 2x512] flush late-ish so early HBM bandwidth goes to input lead
  - last output group's 4 DMAs split across both HWDGE rings for the drain
Measured rejects: fp8 (3.8e-2 L2 err > 2e-2 gate), 4096-wide input chunks,
single 3D-AP DMA per chunk (f16t3), dual-ring early input, outw=2048 groups —
all slower and/or fragile (input starvation -> HAM re-throttle outliers).
"""

import os

import numpy as np

import concourse.bacc as bacc
import concourse.mybir as mybir
import concourse.tile as tile
from concourse.bass_utils import run_bass_kernel_spmd

N_CORES = 8
B = 131072
D = 512  # contraction dim i (NUM_BASIS)
J = 512  # output dim j (MOTION_DIM)
BC = B // N_CORES  # rows per core
P = 128
KT = D // P  # 4 i-tiles

MODE = os.environ.get("KERNEL_MODE", "f16t2")  # f16t2 | f16t | f16 | bf16 | f32r | f16x3

_DT_IN = {
    "f16": mybir.dt.float16,
    "f16t": mybir.dt.float16,
    "f16t2": mybir.dt.float16,
    "bf16": mybir.dt.bfloat16,
    "f32r": mybir.dt.float32r,
    "f16x3": mybir.dt.float16,
}

_compiled = {}
LAST_RESULTS = None  # BassKernelResults of the most recent run (for test.py)


def _np_in_dtype(mode):
    if mode in ("f16", "f16t", "f16t2", "f16t3", "f16x3"):
        return np.float16
    if mode == "bf16":
        import ml_dtypes

        return ml_dtypes.bfloat16
    return np.float32


def _build(mode, bc=BC, chunk=4096, ob=4):
    dt_in = _DT_IN[mode]
    hilo = mode.endswith("x3")
    nc = bacc.Bacc()
    if hilo:
        inpT_hi = nc.dram_tensor("inpT_hi", [D, bc], dt_in, kind="ExternalInput")
        inpT_lo = nc.dram_tensor("inpT_lo", [D, bc], dt_in, kind="ExternalInput")
        qT_hi = nc.dram_tensor("qT_hi", [D, J], dt_in, kind="ExternalInput")
        qT_lo = nc.dram_tensor("qT_lo", [D, J], dt_in, kind="ExternalInput")
        in_drams = [inpT_hi, inpT_lo]
        q_drams = [qT_hi, qT_lo]
    else:
        inpT = nc.dram_tensor("inpT", [D, bc], dt_in, kind="ExternalInput")
        qT = nc.dram_tensor("qT", [D, J], dt_in, kind="ExternalInput")
        in_drams = [inpT]
        q_drams = [qT]
    out = nc.dram_tensor("out", [bc, J], mybir.dt.float32, kind="ExternalOutput")

    BCk = bc
    CHUNK = chunk  # b-columns fetched per supertile DMA (1 MB in fp16)
    OB = ob  # b-tiles batched per output DMA instruction

    # Output viewed as [p, ob-groups, j] so one DMA stores OB b-tiles.
    out3 = out.rearrange("(g ob p) j -> g p ob j", p=P, ob=OB)

    with tile.TileContext(nc) as tc:
        with (
            tc.tile_pool(name="qpool", bufs=1) as qpool,
            tc.tile_pool(name="inpool", bufs=2) as inpool,
            tc.tile_pool(name="outpool", bufs=3) as outpool,
            tc.tile_pool(name="psum", bufs=7, space="PSUM") as psum_pool,
        ):
            # Q.T tiles [i=128, j=512], static for the whole kernel.
            qts = []
            for qi, qd in enumerate(q_drams):
                for it in range(KT):
                    qt_t = qpool.tile([P, J], dt_in, tag=f"qt{qi}_{it}")
                    nc.sync.dma_start(out=qt_t[:], in_=qd[it * P : (it + 1) * P, :])
                    qts.append(qt_t)

            ot = None
            for chunk in range(BCk // CHUNK):
                csl = slice(chunk * CHUNK, (chunk + 1) * CHUNK)
                sups = []  # supertiles per (input, i-tile)
                for ii, ind in enumerate(in_drams):
                    for it in range(KT):
                        sup = inpool.tile([P, CHUNK], dt_in, tag=f"sup{ii}_{it}")
                        # input loads ride the ACT HWDGE ring; output the SP ring
                        nc.scalar.dma_start(
                            out=sup[:], in_=ind[it * P : (it + 1) * P, csl]
                        )
                        sups.append(sup)
                for bt in range(CHUNK // P):
                    bsl = slice(bt * P, (bt + 1) * P)
                    ps = psum_pool.tile([P, J], mybir.dt.float32, tag="ps")
                    if hilo:
                        # out = hi@Qhi + hi@Qlo + lo@Qhi  (drop lo@Qlo)
                        passes = [(0, 0), (0, 1), (1, 0)]
                    else:
                        passes = [(0, 0)]
                    n_mm = len(passes) * KT
                    mm = 0
                    for ii, qi in passes:
                        for it in range(KT):
                            nc.tensor.matmul(
                                ps[:],
                                sups[ii * KT + it][:, bsl],
                                qts[qi * KT + it][:],
                                start=(mm == 0),
                                stop=(mm == n_mm - 1),
                            )
                            mm += 1
                    gbt = chunk * (CHUNK // P) + bt  # global b-tile index
                    if gbt % OB == 0:
                        ot = outpool.tile([P, OB, J], mybir.dt.float32, tag="ot")
                    # split PSUM->SBUF copies across DVE and ACT
                    if gbt % 2 == 0:
                        nc.vector.tensor_copy(out=ot[:, gbt % OB, :], in_=ps[:])
                    else:
                        nc.scalar.copy(out=ot[:, gbt % OB, :], in_=ps[:])
                    if gbt % OB == OB - 1:
                        nc.sync.dma_start(out=out3[gbt // OB], in_=ot[:])
    nc.compile()
    return nc


def _build_t(mode, bc=BC, chunk=2048, outw=4096, warmup_mms=0):
    """Transposed-output variant: PSUM holds [j, b] tiles (stationary = Q.T
    128x128 blocks, moving = inpT [i, b] slices), output written as
    outT [J, bc] fp16 with wide per-partition runs, host transposes back.
    Halves output HBM traffic and keeps DMA packets large (>= 4 KB)."""
    dt_in = _DT_IN[mode]
    assert dt_in == mybir.dt.float16
    nc = bacc.Bacc()
    inpT = nc.dram_tensor("inpT", [D, bc], dt_in, kind="ExternalInput")
    qT = nc.dram_tensor("qT", [D, J], dt_in, kind="ExternalInput")
    outT = nc.dram_tensor("outT", [J, bc], mybir.dt.float16, kind="ExternalOutput")

    NB = 512  # moving free dim per matmul (one PSUM bank of fp32)
    JT = J // P  # 4 j-tiles

    # Input chunk schedule: uniform chunks (leading small chunk measured worse).
    plan = []
    rem = bc
    while rem > 0:
        c = min(chunk, rem)
        plan.append(c)
        rem -= c

    # Output group schedule: small groups at both ends (early first store,
    # short final flush), wide in the middle for large DMA packets.
    ow_plan = []
    rem = bc
    if bc >= 4 * outw:
        for c in (1024, 1024, 2048):
            ow_plan.append(c)
            rem -= c
    tail = [1024, 1024, 2048] if bc >= 4 * outw else []
    rem -= sum(tail)
    while rem > 0:
        c = min(outw, rem)
        ow_plan.append(c)
        rem -= c
    ow_plan.extend(reversed(tail))
    assert sum(ow_plan) == bc and all(w % 512 == 0 for w in ow_plan)
    # column index -> (group_idx, offset, width)
    col2grp = {}
    base = 0
    for gi, w in enumerate(ow_plan):
        for off in range(0, w, 512):
            col2grp[base + off] = (gi, off, w)
        base += w
    grp_base = {}
    base = 0
    for gi, w in enumerate(ow_plan):
        grp_base[gi] = base
        base += w

    with tile.TileContext(nc) as tc:
        with (
            tc.tile_pool(name="qpool", bufs=1) as qpool,
            tc.tile_pool(name="inpool", bufs=3) as inpool,
            tc.tile_pool(name="outpool", bufs=2) as outpool,
            tc.tile_pool(name="psum", bufs=8, space="PSUM") as psum_pool,
            tc.tile_pool(name="warm", bufs=1) as warm_pool,
            tc.tile_pool(name="warmps", bufs=1, space="PSUM") as warmps_pool,
        ):
            # Q.T rows for i-tile `it`: [128i, 512j]; stationary blocks are
            # 128-column slices qts[it][:, jt*128:(jt+1)*128]. (Dedicated
            # contiguous [128,128] weight tiles measured WORSE: 137.2us.)
            qts = []
            for it in range(KT):
                qt_t = qpool.tile([P, J], dt_in, tag=f"qt{it}")
                nc.gpsimd.dma_start(out=qt_t[:], in_=qT[it * P : (it + 1) * P, :])
                qts.append(qt_t)

            if warmup_mms:
                # Warmup matmuls on the (tiny, early-arriving) qT tiles: keeps
                # the PE HAM busy while the first input chunk streams in, so
                # real matmuls start un-throttled. Result bank is never read.
                wps = warmps_pool.tile([P, NB], mybir.dt.float32, tag="wps")
                for wi in range(warmup_mms):
                    nc.tensor.matmul(
                        wps[:],
                        qts[0][:, :P],
                        qts[0][:],
                        start=(wi == 0),
                        stop=(wi == warmup_mms - 1),
                    )

            ots = [None] * JT
            col_base = 0
            for chunk_i, csz in enumerate(plan):
                csl = slice(col_base, col_base + csz)
                sups = []
                for it in range(KT):
                    sup = inpool.tile([P, csz], dt_in, tag=f"sup{it}")
                    nc.scalar.dma_start(
                        out=sup[:], in_=inpT[it * P : (it + 1) * P, csl]
                    )
                    sups.append(sup)
                for bn in range(csz // NB):
                    col0 = col_base + bn * NB
                    gi, goff, gw = col2grp[col0]
                    if goff == 0:
                        for jt in range(JT):
                            ots[jt] = outpool.tile(
                                [P, outw],
                                mybir.dt.float16,
                                tag=f"ot{jt}",
                                name=f"ot{jt}",
                            )
                    osl = slice(goff, goff + NB)
                    bsl = slice(bn * NB, (bn + 1) * NB)
                    for jt in range(JT):
                        ps = psum_pool.tile([P, NB], mybir.dt.float32, tag="ps")
                        for it in range(KT):
                            nc.tensor.matmul(
                                ps[:],
                                qts[it][:, jt * P : (jt + 1) * P],
                                sups[it][:, bsl],
                                start=(it == 0),
                                stop=(it == KT - 1),
                            )
                        if jt % 2 == 0:
                            nc.vector.tensor_copy(out=ots[jt][:, osl], in_=ps[:])
                        else:
                            nc.scalar.copy(out=ots[jt][:, osl], in_=ps[:])
                    if goff + NB == gw:
                        g0 = grp_base[gi]
                        for jt in range(JT):
                            nc.sync.dma_start(
                                out=outT[jt * P : (jt + 1) * P, g0 : g0 + gw],
                                in_=ots[jt][:, :gw],
                            )
                col_base += csz
    nc.compile()
    return nc


def _build_t2(
    mode="f16t2",
    bc=BC,
    warmup=int(os.environ.get("KW", "26")),
    in_plan=None,
    ow_plan=None,
    inbufs=int(os.environ.get("KIB", "5")),
    outbufs=int(os.environ.get("KOB", "2")),
):
    """v2 of the transposed-output kernel. Changes vs _build_t (all aimed at
    ramp-in/ramp-out; steady state was already at the 216 ns/MM PE floor):
      - warmup matmuls on a DVE-memset zero tile, issued before any DMA lands,
        so the PE HAM un-throttles (K=8/8) before real matmuls start
      - Q.T loaded with ONE sync-ring (HWDGE) DMA instead of 4 SWDGE DMAs
        (landed at 10.4us before; sync ring is idle during ramp)
      - small leading input chunks so the first real matmul starts ~4us earlier
      - it-outer matmul order: matmuls for i-tile `it` only need sup[it], so
        compute starts as soon as the first quarter-chunk lands
      - outpool bufs=3 (group buffer recycle caused a 1.6us mid-stream stall)
      - output group plan ends small (512) to shorten the drain tail
    """
    dt_in = mybir.dt.float16
    nc = bacc.Bacc()
    inpT = nc.dram_tensor("inpT", [D, bc], dt_in, kind="ExternalInput")
    qT = nc.dram_tensor("qT", [D, J], dt_in, kind="ExternalInput")
    outT = nc.dram_tensor("outT", [J, bc], mybir.dt.float16, kind="ExternalOutput")

    NB = 512  # moving free dim per matmul (one PSUM bank of fp32)
    JT = J // P  # 4 j-tiles
    INW = 2048  # inpool tile width (chunks are subviews)

    if in_plan is None:
        in_plan = [512, 512, 1024] + [2048] * 7
    assert sum(in_plan) == bc and all(c % NB == 0 and c <= INW for c in in_plan)
    if ow_plan is None:
        ow_plan = [4096, 4096, 4096, 2048, 1024, 512, 512]
    assert sum(ow_plan) == bc and all(w % NB == 0 for w in ow_plan)
    outw = max(ow_plan)
    n_grp = len(ow_plan)

    # column index -> (group_idx, offset, width)
    col2grp = {}
    grp_base = {}
    base = 0
    for gi, w in enumerate(ow_plan):
        grp_base[gi] = base
        for off in range(0, w, NB):
            col2grp[base + off] = (gi, off, w)
        base += w

    # Q.T as [128, k, j]: one DMA for all 4 i-tiles
    qT2 = qT.rearrange("(k p) j -> p k j", p=P)

    with tile.TileContext(nc) as tc:
        with (
            tc.tile_pool(name="warm", bufs=1) as warm_pool,
            tc.tile_pool(name="qpool", bufs=1) as qpool,
            tc.tile_pool(name="inpool", bufs=inbufs) as inpool,
            tc.tile_pool(name="outpool", bufs=outbufs) as outpool,
            tc.tile_pool(name="psum", bufs=8, space="PSUM") as psum_pool,
        ):
            # --- PE warmup: matmuls on a zeroed tile, no DMA dependency ---
            if warmup:
                wt = warm_pool.tile([P, P], dt_in, tag="wt")
                nc.vector.memset(wt[:], 0.0)
                wps = psum_pool.tile([P, NB], mybir.dt.float32, tag="ps")
                for wi in range(warmup):
                    nc.tensor.matmul(
                        wps[:, :P],
                        wt[:],
                        wt[:],
                        start=(wi == 0),
                        stop=(wi == warmup - 1),
                    )

            # --- Q.T: per-i-tile sync-ring DMAs so the first matmul only
            # waits for the first 128 KB, not the full 512 KB ---
            qt = qpool.tile([P, KT, J], dt_in, tag="qt")
            for it in range(KT):
                nc.sync.dma_start(out=qt[:, it, :], in_=qT2[:, it, :])

            def qw(it, jt):  # stationary [128i, 128j] block
                return qt[:, it, jt * P : (jt + 1) * P]

            ots = [None] * JT
            col_base = 0
            for chunk_i, csz in enumerate(in_plan):
                csl = slice(col_base, col_base + csz)
                # NOTE: routing early chunks over the sync ring as well
                # (dual-ring input) measured ~2us WORSE across 6 runs.
                ieng = nc.scalar
                sups = []
                for it in range(KT):
                    sup = inpool.tile([P, INW], dt_in, tag=f"sup{it}")
                    ieng.dma_start(
                        out=sup[:, :csz], in_=inpT[it * P : (it + 1) * P, csl]
                    )
                    sups.append(sup)
                for bn in range(csz // NB):
                    col0 = col_base + bn * NB
                    gi, goff, gw = col2grp[col0]
                    if goff == 0:
                        for jt in range(JT):
                            ots[jt] = outpool.tile(
                                [P, outw],
                                mybir.dt.float16,
                                tag=f"ot{jt}",
                                name=f"ot{jt}",
                            )
                    osl = slice(goff, goff + NB)
                    bsl = slice(bn * NB, (bn + 1) * NB)
                    pss = [
                        psum_pool.tile([P, NB], mybir.dt.float32, tag="ps", name=f"ps{jt}")
                        for jt in range(JT)
                    ]
                    # it-outer: matmuls for i-tile `it` need only sups[it]
                    for it in range(KT):
                        for jt in range(JT):
                            nc.tensor.matmul(
                                pss[jt][:],
                                qw(it, jt),
                                sups[it][:, bsl],
                                start=(it == 0),
                                stop=(it == KT - 1),
                            )
                    for jt in range(JT):
                        if jt % 2 == 0:
                            nc.vector.tensor_copy(out=ots[jt][:, osl], in_=pss[jt][:])
                        else:
                            nc.scalar.copy(out=ots[jt][:, osl], in_=pss[jt][:])
                    if goff + NB == gw:
                        g0 = grp_base[gi]
                        for jt in range(JT):
                            # final group: input ring is idle, split across both
                            eng = (
                                nc.scalar
                                if (gi == n_grp - 1 and jt % 2 == 1)
                                else nc.sync
                            )
                            eng.dma_start(
                                out=outT[jt * P : (jt + 1) * P, g0 : g0 + gw],
                                in_=ots[jt][:, :gw],
                            )
                col_base += csz
    nc.compile()
    return nc


def _build_t3(
    mode="f16t3",
    bc=BC,
    warmup=int(os.environ.get("KW", "26")),
    in_plan=None,
    ow_plan=None,
    inbufs=int(os.environ.get("KIB", "5")),
    outbufs=int(os.environ.get("KOB", "2")),
):
    """v3 of the transposed-output kernel. vs _build_t2:
      - each input chunk is ONE 3D-AP DMA (all 4 i-tiles) on the sync ring:
        4x less HWDGE descriptor-gen time on the issuing engine, and input
        DGE no longer shares a queue with PSUM copies
      - qt + output groups ride the scalar(ACT) ring
      - PSUM copies rebalanced: jt0..2 on DVE, jt3 on ACT (ACT also runs
        the output DGE bursts)
    """
    dt_in = mybir.dt.float16
    nc = bacc.Bacc()
    inpT = nc.dram_tensor("inpT", [D, bc], dt_in, kind="ExternalInput")
    qT = nc.dram_tensor("qT", [D, J], dt_in, kind="ExternalInput")
    outT = nc.dram_tensor("outT", [J, bc], mybir.dt.float16, kind="ExternalOutput")

    NB = 512
    JT = J // P
    INW = 2048

    if in_plan is None:
        in_plan = [512, 512, 1024] + [2048] * 7
    assert sum(in_plan) == bc and all(c % NB == 0 and c <= INW for c in in_plan)
    if ow_plan is None:
        ow_plan = [4096, 4096, 4096, 2048, 1024, 512, 512]
    assert sum(ow_plan) == bc and all(w % NB == 0 for w in ow_plan)
    outw = max(ow_plan)
    n_grp = len(ow_plan)

    col2grp = {}
    grp_base = {}
    base = 0
    for gi, w in enumerate(ow_plan):
        grp_base[gi] = base
        for off in range(0, w, NB):
            col2grp[base + off] = (gi, off, w)
        base += w

    qT2 = qT.rearrange("(k p) j -> p k j", p=P)
    inpT3 = inpT.rearrange("(k p) b -> p k b", p=P)

    with tile.TileContext(nc) as tc:
        with (
            tc.tile_pool(name="warm", bufs=1) as warm_pool,
            tc.tile_pool(name="qpool", bufs=1) as qpool,
            tc.tile_pool(name="inpool", bufs=inbufs) as inpool,
            tc.tile_pool(name="outpool", bufs=outbufs) as outpool,
            tc.tile_pool(name="psum", bufs=8, space="PSUM") as psum_pool,
        ):
            if warmup:
                wt = warm_pool.tile([P, P], dt_in, tag="wt")
                nc.vector.memset(wt[:], 0.0)
                wps = psum_pool.tile([P, NB], mybir.dt.float32, tag="ps")
                for wi in range(warmup):
                    nc.tensor.matmul(
                        wps[:, :P],
                        wt[:],
                        wt[:],
                        start=(wi == 0),
                        stop=(wi == warmup - 1),
                    )

            qt = qpool.tile([P, KT, J], dt_in, tag="qt")
            for it in range(KT):
                nc.scalar.dma_start(out=qt[:, it, :], in_=qT2[:, it, :])

            def qw(it, jt):
                return qt[:, it, jt * P : (jt + 1) * P]

            ots = [None] * JT
            col_base = 0
            for chunk_i, csz in enumerate(in_plan):
                csl = slice(col_base, col_base + csz)
                sup = inpool.tile([P, KT, INW], dt_in, tag="sup")
                nc.sync.dma_start(out=sup[:, :, :csz], in_=inpT3[:, :, csl])
                for bn in range(csz // NB):
                    col0 = col_base + bn * NB
                    gi, goff, gw = col2grp[col0]
                    if goff == 0:
                        for jt in range(JT):
                            ots[jt] = outpool.tile(
                                [P, outw],
                                mybir.dt.float16,
                                tag=f"ot{jt}",
                                name=f"ot{jt}",
                            )
                    osl = slice(goff, goff + NB)
                    bsl = slice(bn * NB, (bn + 1) * NB)
                    pss = [
                        psum_pool.tile([P, NB], mybir.dt.float32, tag="ps", name=f"ps{jt}")
                        for jt in range(JT)
                    ]
                    for it in range(KT):
                        for jt in range(JT):
                            nc.tensor.matmul(
                                pss[jt][:],
                                qw(it, jt),
                                sup[:, it, bsl],
                                start=(it == 0),
                                stop=(it == KT - 1),
                            )
                    for jt in range(JT):
                        if jt < 3:
                            nc.vector.tensor_copy(out=ots[jt][:, osl], in_=pss[jt][:])
                        else:
                            nc.scalar.copy(out=ots[jt][:, osl], in_=pss[jt][:])
                    if goff + NB == gw:
                        g0 = grp_base[gi]
                        for jt in range(JT):
                            eng = (
                                nc.sync
                                if (gi == n_grp - 1 and jt % 2 == 1)
                                else nc.scalar
                            )
                            eng.dma_start(
                                out=outT[jt * P : (jt + 1) * P, g0 : g0 + gw],
                                in_=ots[jt][:, :gw],
                            )
                col_base += csz
    nc.compile()
    return nc


def _get_nc(mode):
    if mode not in _compiled:
        if mode == "f16t3":
            _compiled[mode] = _build_t3(mode)
        elif mode == "f16t2":
            _compiled[mode] = _build_t2(mode)
        elif mode == "f16t":
            _compiled[mode] = _build_t(mode)
        else:
            _compiled[mode] = _build(mode)
    return _compiled[mode]


def kernel(inp: np.ndarray, weight: np.ndarray) -> np.ndarray:
    global LAST_RESULTS
    mode = MODE
    nc = _get_nc(mode)

    w = np.asarray(weight, dtype=np.float32) + np.float32(1e-8)
    Q = np.linalg.qr(w)[0].astype(np.float32)  # [J, D] == [512, 512]
    np_dt = _np_in_dtype(mode)

    inp = np.asarray(inp, dtype=np.float32)
    inpT = inp.T  # [D, B] view

    QT = Q.T  # QT[i, j] = Q[j, i]
    in_maps = []
    if mode.endswith("x3"):
        qt_hi = QT.astype(np_dt)
        qt_lo = (QT - qt_hi.astype(np.float32)).astype(np_dt)
        for c in range(N_CORES):
            sl = inpT[:, c * BC : (c + 1) * BC].astype(np.float32)
            hi = sl.astype(np_dt)
            lo = (sl - hi.astype(np.float32)).astype(np_dt)
            in_maps.append(
                {"inpT_hi": hi, "inpT_lo": lo, "qT_hi": qt_hi, "qT_lo": qt_lo}
            )
    else:
        qt16 = np.ascontiguousarray(QT).astype(np_dt)
        for c in range(N_CORES):
            in_maps.append(
                {"inpT": inpT[:, c * BC : (c + 1) * BC].astype(np_dt), "qT": qt16}
            )

    # First execution of a freshly compiled NEFF occasionally dies with
    # NRT_EXEC_UNIT_UNRECOVERABLE (transient, esp. with profiling on);
    # a straight retry has always succeeded.
    last_exc = None
    for _attempt in range(3):
        try:
            res = run_bass_kernel_spmd(nc, in_maps, list(range(N_CORES)))
            break
        except Exception as e:  # noqa: BLE001
            last_exc = e
            import time as _time

            _time.sleep(2.0)
    else:
        raise last_exc
    LAST_RESULTS = res
    if mode in ("f16t", "f16t2", "f16t3"):
        out = np.empty((B, J), dtype=np.float32)
        for c in range(N_CORES):
            # outT [J, BC] fp16 -> out rows [c*BC:(c+1)*BC] fp32
            out[c * BC : (c + 1) * BC, :] = res.results[c]["outT"].T
        return out
    return np.concatenate([res.results[c]["out"] for c in range(N_CORES)], axis=0)



# revision 32
# speedup vs baseline: 1.0005x; 1.0005x over previous
"""Trainium2 Bass kernel for nn_Dictionary (vq_codebook): out = inp @ Q.T, Q from QR(weight+1e-8).

Strategy (per sharding_hint): data-parallel over batch B=131072 across 8 cores
(16384 rows each); Q.T replicated on every core (QR is tiny, computed on host).

Default mode "f16t2" (_build_t2): host transposes inp so the contraction dim i
lands on SBUF partitions with contiguous DMAs, operands in fp16 (1 cyc/row on
the PE, fp32 PSUM accumulation -> rel L2 err ~3.6e-4). Per core: stationary =
128x128 blocks of Q.T, moving = [128i, 512b] slices of inpT chunks, PSUM holds
out.T [128j, 512b] accumulated over 4 i-tiles (it-outer order so matmuls start
as each i-tile lands); DVE/ACT cast-copy PSUM into [128, <=4096] fp16 out.T
group tiles; host transposes back and upcasts to fp32.

The steady-state PE stream is at the fp16 floor (512 matmuls x 215.8 ns =
110.5us; median inter-MM gap = 216 ns), so v2 targets the edges (132.4 ->
~128.5us median):
  - 26 warmup matmuls on a DVE-memset zero tile keep the PE busy from ~7.5us
    (right after the fixed ~7us Tile/NEFF preamble) so the HAM clock gate
    un-throttles to 2.4 GHz around when real data lands (~10.2us)
  - qT rides the sync ring as 4 per-i-tile DMAs (lands ~9.3us); input chunks
    ride the scalar ring with a small leading chunk (512 cols)
  - input prefetch: 10 chunks, inpool bufs=5; output groups [3x4096, 2048,
    1024, 2x512] flush late-ish so early HBM bandwidth goes to input lead
  - last output group's 4 DMAs split across both HWDGE rings for the drain
Measured rejects (each ~1-4us slower and/or fragile — input starvation ->
HAM re-throttle outliers): fp8 (3.8e-2 L2 err > 2e-2 gate), 4096-wide input
chunks, single 3D-AP DMA per chunk (f16t3), dual-ring early input, outw=2048
groups, input-on-sync/output-on-scalar ring swap (f16t4), inpool bufs=6,
outpool bufs=3.
"""

import os

import numpy as np

import concourse.bacc as bacc
import concourse.mybir as mybir
import concourse.tile as tile
from concourse.bass_utils import run_bass_kernel_spmd

N_CORES = 8
B = 131072
D = 512  # contraction dim i (NUM_BASIS)
J = 512  # output dim j (MOTION_DIM)
BC = B // N_CORES  # rows per core
P = 128
KT = D // P  # 4 i-tiles

MODE = os.environ.get("KERNEL_MODE", "f16t2")  # f16t2 | f16t | f16 | bf16 | f32r | f16x3

_DT_IN = {
    "f16": mybir.dt.float16,
    "f16t": mybir.dt.float16,
    "f16t2": mybir.dt.float16,
    "bf16": mybir.dt.bfloat16,
    "f32r": mybir.dt.float32r,
    "f16x3": mybir.dt.float16,
}

_compiled = {}
LAST_RESULTS = None  # BassKernelResults of the most recent run (for test.py)


def _np_in_dtype(mode):
    if mode in ("f16", "f16t", "f16t2", "f16t3", "f16t4", "f16x3"):
        return np.float16
    if mode == "bf16":
        import ml_dtypes

        return ml_dtypes.bfloat16
    return np.float32


def _build(mode, bc=BC, chunk=4096, ob=4):
    dt_in = _DT_IN[mode]
    hilo = mode.endswith("x3")
    nc = bacc.Bacc()
    if hilo:
        inpT_hi = nc.dram_tensor("inpT_hi", [D, bc], dt_in, kind="ExternalInput")
        inpT_lo = nc.dram_tensor("inpT_lo", [D, bc], dt_in, kind="ExternalInput")
        qT_hi = nc.dram_tensor("qT_hi", [D, J], dt_in, kind="ExternalInput")
        qT_lo = nc.dram_tensor("qT_lo", [D, J], dt_in, kind="ExternalInput")
        in_drams = [inpT_hi, inpT_lo]
        q_drams = [qT_hi, qT_lo]
    else:
        inpT = nc.dram_tensor("inpT", [D, bc], dt_in, kind="ExternalInput")
        qT = nc.dram_tensor("qT", [D, J], dt_in, kind="ExternalInput")
        in_drams = [inpT]
        q_drams = [qT]
    out = nc.dram_tensor("out", [bc, J], mybir.dt.float32, kind="ExternalOutput")

    BCk = bc
    CHUNK = chunk  # b-columns fetched per supertile DMA (1 MB in fp16)
    OB = ob  # b-tiles batched per output DMA instruction

    # Output viewed as [p, ob-groups, j] so one DMA stores OB b-tiles.
    out3 = out.rearrange("(g ob p) j -> g p ob j", p=P, ob=OB)

    with tile.TileContext(nc) as tc:
        with (
            tc.tile_pool(name="qpool", bufs=1) as qpool,
            tc.tile_pool(name="inpool", bufs=2) as inpool,
            tc.tile_pool(name="outpool", bufs=3) as outpool,
            tc.tile_pool(name="psum", bufs=7, space="PSUM") as psum_pool,
        ):
            # Q.T tiles [i=128, j=512], static for the whole kernel.
            qts = []
            for qi, qd in enumerate(q_drams):
                for it in range(KT):
                    qt_t = qpool.tile([P, J], dt_in, tag=f"qt{qi}_{it}")
                    nc.sync.dma_start(out=qt_t[:], in_=qd[it * P : (it + 1) * P, :])
                    qts.append(qt_t)

            ot = None
            for chunk in range(BCk // CHUNK):
                csl = slice(chunk * CHUNK, (chunk + 1) * CHUNK)
                sups = []  # supertiles per (input, i-tile)
                for ii, ind in enumerate(in_drams):
                    for it in range(KT):
                        sup = inpool.tile([P, CHUNK], dt_in, tag=f"sup{ii}_{it}")
                        # input loads ride the ACT HWDGE ring; output the SP ring
                        nc.scalar.dma_start(
                            out=sup[:], in_=ind[it * P : (it + 1) * P, csl]
                        )
                        sups.append(sup)
                for bt in range(CHUNK // P):
                    bsl = slice(bt * P, (bt + 1) * P)
                    ps = psum_pool.tile([P, J], mybir.dt.float32, tag="ps")
                    if hilo:
                        # out = hi@Qhi + hi@Qlo + lo@Qhi  (drop lo@Qlo)
                        passes = [(0, 0), (0, 1), (1, 0)]
                    else:
                        passes = [(0, 0)]
                    n_mm = len(passes) * KT
                    mm = 0
                    for ii, qi in passes:
                        for it in range(KT):
                            nc.tensor.matmul(
                                ps[:],
                                sups[ii * KT + it][:, bsl],
                                qts[qi * KT + it][:],
                                start=(mm == 0),
                                stop=(mm == n_mm - 1),
                            )
                            mm += 1
                    gbt = chunk * (CHUNK // P) + bt  # global b-tile index
                    if gbt % OB == 0:
                        ot = outpool.tile([P, OB, J], mybir.dt.float32, tag="ot")
                    # split PSUM->SBUF copies across DVE and ACT
                    if gbt % 2 == 0:
                        nc.vector.tensor_copy(out=ot[:, gbt % OB, :], in_=ps[:])
                    else:
                        nc.scalar.copy(out=ot[:, gbt % OB, :], in_=ps[:])
                    if gbt % OB == OB - 1:
                        nc.sync.dma_start(out=out3[gbt // OB], in_=ot[:])
    nc.compile()
    return nc


def _build_t(mode, bc=BC, chunk=2048, outw=4096, warmup_mms=0):
    """Transposed-output variant: PSUM holds [j, b] tiles (stationary = Q.T
    128x128 blocks, moving = inpT [i, b] slices), output written as
    outT [J, bc] fp16 with wide per-partition runs, host transposes back.
    Halves output HBM traffic and keeps DMA packets large (>= 4 KB)."""
    dt_in = _DT_IN[mode]
    assert dt_in == mybir.dt.float16
    nc = bacc.Bacc()
    inpT = nc.dram_tensor("inpT", [D, bc], dt_in, kind="ExternalInput")
    qT = nc.dram_tensor("qT", [D, J], dt_in, kind="ExternalInput")
    outT = nc.dram_tensor("outT", [J, bc], mybir.dt.float16, kind="ExternalOutput")

    NB = 512  # moving free dim per matmul (one PSUM bank of fp32)
    JT = J // P  # 4 j-tiles

    # Input chunk schedule: uniform chunks (leading small chunk measured worse).
    plan = []
    rem = bc
    while rem > 0:
        c = min(chunk, rem)
        plan.append(c)
        rem -= c

    # Output group schedule: small groups at both ends (early first store,
    # short final flush), wide in the middle for large DMA packets.
    ow_plan = []
    rem = bc
    if bc >= 4 * outw:
        for c in (1024, 1024, 2048):
            ow_plan.append(c)
            rem -= c
    tail = [1024, 1024, 2048] if bc >= 4 * outw else []
    rem -= sum(tail)
    while rem > 0:
        c = min(outw, rem)
        ow_plan.append(c)
        rem -= c
    ow_plan.extend(reversed(tail))
    assert sum(ow_plan) == bc and all(w % 512 == 0 for w in ow_plan)
    # column index -> (group_idx, offset, width)
    col2grp = {}
    base = 0
    for gi, w in enumerate(ow_plan):
        for off in range(0, w, 512):
            col2grp[base + off] = (gi, off, w)
        base += w
    grp_base = {}
    base = 0
    for gi, w in enumerate(ow_plan):
        grp_base[gi] = base
        base += w

    with tile.TileContext(nc) as tc:
        with (
            tc.tile_pool(name="qpool", bufs=1) as qpool,
            tc.tile_pool(name="inpool", bufs=3) as inpool,
            tc.tile_pool(name="outpool", bufs=2) as outpool,
            tc.tile_pool(name="psum", bufs=8, space="PSUM") as psum_pool,
            tc.tile_pool(name="warm", bufs=1) as warm_pool,
            tc.tile_pool(name="warmps", bufs=1, space="PSUM") as warmps_pool,
        ):
            # Q.T rows for i-tile `it`: [128i, 512j]; stationary blocks are
            # 128-column slices qts[it][:, jt*128:(jt+1)*128]. (Dedicated
            # contiguous [128,128] weight tiles measured WORSE: 137.2us.)
            qts = []
            for it in range(KT):
                qt_t = qpool.tile([P, J], dt_in, tag=f"qt{it}")
                nc.gpsimd.dma_start(out=qt_t[:], in_=qT[it * P : (it + 1) * P, :])
                qts.append(qt_t)

            if warmup_mms:
                # Warmup matmuls on the (tiny, early-arriving) qT tiles: keeps
                # the PE HAM busy while the first input chunk streams in, so
                # real matmuls start un-throttled. Result bank is never read.
                wps = warmps_pool.tile([P, NB], mybir.dt.float32, tag="wps")
                for wi in range(warmup_mms):
                    nc.tensor.matmul(
                        wps[:],
                        qts[0][:, :P],
                        qts[0][:],
                        start=(wi == 0),
                        stop=(wi == warmup_mms - 1),
                    )

            ots = [None] * JT
            col_base = 0
            for chunk_i, csz in enumerate(plan):
                csl = slice(col_base, col_base + csz)
                sups = []
                for it in range(KT):
                    sup = inpool.tile([P, csz], dt_in, tag=f"sup{it}")
                    nc.scalar.dma_start(
                        out=sup[:], in_=inpT[it * P : (it + 1) * P, csl]
                    )
                    sups.append(sup)
                for bn in range(csz // NB):
                    col0 = col_base + bn * NB
                    gi, goff, gw = col2grp[col0]
                    if goff == 0:
                        for jt in range(JT):
                            ots[jt] = outpool.tile(
                                [P, outw],
                                mybir.dt.float16,
                                tag=f"ot{jt}",
                                name=f"ot{jt}",
                            )
                    osl = slice(goff, goff + NB)
                    bsl = slice(bn * NB, (bn + 1) * NB)
                    for jt in range(JT):
                        ps = psum_pool.tile([P, NB], mybir.dt.float32, tag="ps")
                        for it in range(KT):
                            nc.tensor.matmul(
                                ps[:],
                                qts[it][:, jt * P : (jt + 1) * P],
                                sups[it][:, bsl],
                                start=(it == 0),
                                stop=(it == KT - 1),
                            )
                        if jt % 2 == 0:
                            nc.vector.tensor_copy(out=ots[jt][:, osl], in_=ps[:])
                        else:
                            nc.scalar.copy(out=ots[jt][:, osl], in_=ps[:])
                    if goff + NB == gw:
                        g0 = grp_base[gi]
                        for jt in range(JT):
                            nc.sync.dma_start(
                                out=outT[jt * P : (jt + 1) * P, g0 : g0 + gw],
                                in_=ots[jt][:, :gw],
                            )
                col_base += csz
    nc.compile()
    return nc


def _build_t2(
    mode="f16t2",
    bc=BC,
    warmup=int(os.environ.get("KW", "26")),
    in_plan=None,
    ow_plan=None,
    inbufs=int(os.environ.get("KIB", "5")),
    outbufs=int(os.environ.get("KOB", "2")),
    swap_rings=False,
):
    """v2 of the transposed-output kernel. Changes vs _build_t (all aimed at
    ramp-in/ramp-out; steady state was already at the 216 ns/MM PE floor):
      - warmup matmuls on a DVE-memset zero tile, issued before any DMA lands,
        so the PE HAM un-throttles (K=8/8) before real matmuls start
      - Q.T loaded with ONE sync-ring (HWDGE) DMA instead of 4 SWDGE DMAs
        (landed at 10.4us before; sync ring is idle during ramp)
      - small leading input chunks so the first real matmul starts ~4us earlier
      - it-outer matmul order: matmuls for i-tile `it` only need sup[it], so
        compute starts as soon as the first quarter-chunk lands
      - outpool bufs=3 (group buffer recycle caused a 1.6us mid-stream stall)
      - output group plan ends small (512) to shorten the drain tail
    """
    dt_in = mybir.dt.float16
    nc = bacc.Bacc()
    inpT = nc.dram_tensor("inpT", [D, bc], dt_in, kind="ExternalInput")
    qT = nc.dram_tensor("qT", [D, J], dt_in, kind="ExternalInput")
    outT = nc.dram_tensor("outT", [J, bc], mybir.dt.float16, kind="ExternalOutput")

    NB = 512  # moving free dim per matmul (one PSUM bank of fp32)
    JT = J // P  # 4 j-tiles
    INW = 2048  # inpool tile width (chunks are subviews)

    if in_plan is None:
        in_plan = [512, 512, 1024] + [2048] * 7
    assert sum(in_plan) == bc and all(c % NB == 0 and c <= INW for c in in_plan)
    if ow_plan is None:
        ow_plan = [4096, 4096, 4096, 2048, 1024, 512, 512]
    assert sum(ow_plan) == bc and all(w % NB == 0 for w in ow_plan)
    outw = max(ow_plan)
    n_grp = len(ow_plan)

    # column index -> (group_idx, offset, width)
    col2grp = {}
    grp_base = {}
    base = 0
    for gi, w in enumerate(ow_plan):
        grp_base[gi] = base
        for off in range(0, w, NB):
            col2grp[base + off] = (gi, off, w)
        base += w

    # Q.T as [128, k, j]: one DMA for all 4 i-tiles
    qT2 = qT.rearrange("(k p) j -> p k j", p=P)

    with tile.TileContext(nc) as tc:
        with (
            tc.tile_pool(name="warm", bufs=1) as warm_pool,
            tc.tile_pool(name="qpool", bufs=1) as qpool,
            tc.tile_pool(name="inpool", bufs=inbufs) as inpool,
            tc.tile_pool(name="outpool", bufs=outbufs) as outpool,
            tc.tile_pool(name="psum", bufs=8, space="PSUM") as psum_pool,
        ):
            # --- PE warmup: matmuls on a zeroed tile, no DMA dependency ---
            if warmup:
                wt = warm_pool.tile([P, P], dt_in, tag="wt")
                nc.vector.memset(wt[:], 0.0)
                wps = psum_pool.tile([P, NB], mybir.dt.float32, tag="ps")
                for wi in range(warmup):
                    nc.tensor.matmul(
                        wps[:, :P],
                        wt[:],
                        wt[:],
                        start=(wi == 0),
                        stop=(wi == warmup - 1),
                    )

            # swap_rings: input DMAs alone on the sync(SP) queue so their
            # issue is gated only by inpool buffer recycle, not by the ACT
            # copy cadence (observed: input DGE on ACT paced 1 DMA per
            # bn-iter -> prefetch never ran ahead -> ~1.5us stall mid-run);
            # qt + output DMAs then ride the scalar(ACT) queue.
            q_eng = nc.scalar if swap_rings else nc.sync
            in_eng = nc.sync if swap_rings else nc.scalar
            out_eng = nc.scalar if swap_rings else nc.sync
            alt_eng = nc.sync if swap_rings else nc.scalar

            # --- Q.T: per-i-tile DMAs so the first matmul only
            # waits for the first 128 KB, not the full 512 KB ---
            qt = qpool.tile([P, KT, J], dt_in, tag="qt")
            for it in range(KT):
                q_eng.dma_start(out=qt[:, it, :], in_=qT2[:, it, :])

            def qw(it, jt):  # stationary [128i, 128j] block
                return qt[:, it, jt * P : (jt + 1) * P]

            ots = [None] * JT
            col_base = 0
            for chunk_i, csz in enumerate(in_plan):
                csl = slice(col_base, col_base + csz)
                # NOTE: routing early chunks over BOTH rings (dual-ring
                # input) measured ~2us WORSE across 6 runs.
                ieng = in_eng
                sups = []
                for it in range(KT):
                    sup = inpool.tile([P, INW], dt_in, tag=f"sup{it}")
                    ieng.dma_start(
                        out=sup[:, :csz], in_=inpT[it * P : (it + 1) * P, csl]
                    )
                    sups.append(sup)
                for bn in range(csz // NB):
                    col0 = col_base + bn * NB
                    gi, goff, gw = col2grp[col0]
                    if goff == 0:
                        for jt in range(JT):
                            ots[jt] = outpool.tile(
                                [P, outw],
                                mybir.dt.float16,
                                tag=f"ot{jt}",
                                name=f"ot{jt}",
                            )
                    osl = slice(goff, goff + NB)
                    bsl = slice(bn * NB, (bn + 1) * NB)
                    pss = [
                        psum_pool.tile([P, NB], mybir.dt.float32, tag="ps", name=f"ps{jt}")
                        for jt in range(JT)
                    ]
                    # it-outer: matmuls for i-tile `it` need only sups[it]
                    for it in range(KT):
                        for jt in range(JT):
                            nc.tensor.matmul(
                                pss[jt][:],
                                qw(it, jt),
                                sups[it][:, bsl],
                                start=(it == 0),
                                stop=(it == KT - 1),
                            )
                    for jt in range(JT):
                        if jt % 2 == 0:
                            nc.vector.tensor_copy(out=ots[jt][:, osl], in_=pss[jt][:])
                        else:
                            nc.scalar.copy(out=ots[jt][:, osl], in_=pss[jt][:])
                    if goff + NB == gw:
                        g0 = grp_base[gi]
                        for jt in range(JT):
                            # final group: input ring is idle, split across both
                            eng = (
                                alt_eng
                                if (gi == n_grp - 1 and jt % 2 == 1)
                                else out_eng
                            )
                            eng.dma_start(
                                out=outT[jt * P : (jt + 1) * P, g0 : g0 + gw],
                                in_=ots[jt][:, :gw],
                            )
                col_base += csz
    nc.compile()
    return nc


def _build_t3(
    mode="f16t3",
    bc=BC,
    warmup=int(os.environ.get("KW", "26")),
    in_plan=None,
    ow_plan=None,
    inbufs=int(os.environ.get("KIB", "5")),
    outbufs=int(os.environ.get("KOB", "2")),
):
    """v3 of the transposed-output kernel. vs _build_t2:
      - each input chunk is ONE 3D-AP DMA (all 4 i-tiles) on the sync ring:
        4x less HWDGE descriptor-gen time on the issuing engine, and input
        DGE no longer shares a queue with PSUM copies
      - qt + output groups ride the scalar(ACT) ring
      - PSUM copies rebalanced: jt0..2 on DVE, jt3 on ACT (ACT also runs
        the output DGE bursts)
    """
    dt_in = mybir.dt.float16
    nc = bacc.Bacc()
    inpT = nc.dram_tensor("inpT", [D, bc], dt_in, kind="ExternalInput")
    qT = nc.dram_tensor("qT", [D, J], dt_in, kind="ExternalInput")
    outT = nc.dram_tensor("outT", [J, bc], mybir.dt.float16, kind="ExternalOutput")

    NB = 512
    JT = J // P
    INW = 2048

    if in_plan is None:
        in_plan = [512, 512, 1024] + [2048] * 7
    assert sum(in_plan) == bc and all(c % NB == 0 and c <= INW for c in in_plan)
    if ow_plan is None:
        ow_plan = [4096, 4096, 4096, 2048, 1024, 512, 512]
    assert sum(ow_plan) == bc and all(w % NB == 0 for w in ow_plan)
    outw = max(ow_plan)
    n_grp = len(ow_plan)

    col2grp = {}
    grp_base = {}
    base = 0
    for gi, w in enumerate(ow_plan):
        grp_base[gi] = base
        for off in range(0, w, NB):
            col2grp[base + off] = (gi, off, w)
        base += w

    qT2 = qT.rearrange("(k p) j -> p k j", p=P)
    inpT3 = inpT.rearrange("(k p) b -> p k b", p=P)

    with tile.TileContext(nc) as tc:
        with (
            tc.tile_pool(name="warm", bufs=1) as warm_pool,
            tc.tile_pool(name="qpool", bufs=1) as qpool,
            tc.tile_pool(name="inpool", bufs=inbufs) as inpool,
            tc.tile_pool(name="outpool", bufs=outbufs) as outpool,
            tc.tile_pool(name="psum", bufs=8, space="PSUM") as psum_pool,
        ):
            if warmup:
                wt = warm_pool.tile([P, P], dt_in, tag="wt")
                nc.vector.memset(wt[:], 0.0)
                wps = psum_pool.tile([P, NB], mybir.dt.float32, tag="ps")
                for wi in range(warmup):
                    nc.tensor.matmul(
                        wps[:, :P],
                        wt[:],
                        wt[:],
                        start=(wi == 0),
                        stop=(wi == warmup - 1),
                    )

            qt = qpool.tile([P, KT, J], dt_in, tag="qt")
            for it in range(KT):
                nc.scalar.dma_start(out=qt[:, it, :], in_=qT2[:, it, :])

            def qw(it, jt):
                return qt[:, it, jt * P : (jt + 1) * P]

            ots = [None] * JT
            col_base = 0
            for chunk_i, csz in enumerate(in_plan):
                csl = slice(col_base, col_base + csz)
                sup = inpool.tile([P, KT, INW], dt_in, tag="sup")
                nc.sync.dma_start(out=sup[:, :, :csz], in_=inpT3[:, :, csl])
                for bn in range(csz // NB):
                    col0 = col_base + bn * NB
                    gi, goff, gw = col2grp[col0]
                    if goff == 0:
                        for jt in range(JT):
                            ots[jt] = outpool.tile(
                                [P, outw],
                                mybir.dt.float16,
                                tag=f"ot{jt}",
                                name=f"ot{jt}",
                            )
                    osl = slice(goff, goff + NB)
                    bsl = slice(bn * NB, (bn + 1) * NB)
                    pss = [
                        psum_pool.tile([P, NB], mybir.dt.float32, tag="ps", name=f"ps{jt}")
                        for jt in range(JT)
                    ]
                    for it in range(KT):
                        for jt in range(JT):
                            nc.tensor.matmul(
                                pss[jt][:],
                                qw(it, jt),
                                sup[:, it, bsl],
                                start=(it == 0),
                                stop=(it == KT - 1),
                            )
                    for jt in range(JT):
                        if jt < 3:
                            nc.vector.tensor_copy(out=ots[jt][:, osl], in_=pss[jt][:])
                        else:
                            nc.scalar.copy(out=ots[jt][:, osl], in_=pss[jt][:])
                    if goff + NB == gw:
                        g0 = grp_base[gi]
                        for jt in range(JT):
                            eng = (
                                nc.sync
                                if (gi == n_grp - 1 and jt % 2 == 1)
                                else nc.scalar
                            )
                            eng.dma_start(
                                out=outT[jt * P : (jt + 1) * P, g0 : g0 + gw],
                                in_=ots[jt][:, :gw],
                            )
                col_base += csz
    nc.compile()
    return nc


def _get_nc(mode):
    if mode not in _compiled:
        if mode == "f16t4":
            _compiled[mode] = _build_t2(mode, swap_rings=True)
        elif mode == "f16t3":
            _compiled[mode] = _build_t3(mode)
        elif mode == "f16t2":
            _compiled[mode] = _build_t2(mode)
        elif mode == "f16t":
            _compiled[mode] = _build_t(mode)
        else:
            _compiled[mode] = _build(mode)
    return _compiled[mode]


def kernel(inp: np.ndarray, weight: np.ndarray) -> np.ndarray:
    global LAST_RESULTS
    mode = MODE
    nc = _get_nc(mode)

    w = np.asarray(weight, dtype=np.float32) + np.float32(1e-8)
    Q = np.linalg.qr(w)[0].astype(np.float32)  # [J, D] == [512, 512]
    np_dt = _np_in_dtype(mode)

    inp = np.asarray(inp, dtype=np.float32)
    inpT = inp.T  # [D, B] view

    QT = Q.T  # QT[i, j] = Q[j, i]
    in_maps = []
    if mode.endswith("x3"):
        qt_hi = QT.astype(np_dt)
        qt_lo = (QT - qt_hi.astype(np.float32)).astype(np_dt)
        for c in range(N_CORES):
            sl = inpT[:, c * BC : (c + 1) * BC].astype(np.float32)
            hi = sl.astype(np_dt)
            lo = (sl - hi.astype(np.float32)).astype(np_dt)
            in_maps.append(
                {"inpT_hi": hi, "inpT_lo": lo, "qT_hi": qt_hi, "qT_lo": qt_lo}
            )
    else:
        qt16 = np.ascontiguousarray(QT).astype(np_dt)
        for c in range(N_CORES):
            in_maps.append(
                {"inpT": inpT[:, c * BC : (c + 1) * BC].astype(np_dt), "qT": qt16}
            )

    # First execution of a freshly compiled NEFF occasionally dies with
    # NRT_EXEC_UNIT_UNRECOVERABLE (transient, esp. with profiling on);
    # a straight retry has always succeeded.
    last_exc = None
    for _attempt in range(3):
        try:
            res = run_bass_kernel_spmd(nc, in_maps, list(range(N_CORES)))
            break
        except Exception as e:  # noqa: BLE001
            last_exc = e
            import time as _time

            _time.sleep(2.0)
    else:
        raise last_exc
    LAST_RESULTS = res
    if mode in ("f16t", "f16t2", "f16t3", "f16t4"):
        out = np.empty((B, J), dtype=np.float32)
        for c in range(N_CORES):
            # outT [J, BC] fp16 -> out rows [c*BC:(c+1)*BC] fp32
            out[c * BC : (c + 1) * BC, :] = res.results[c]["outT"].T
        return out
    return np.concatenate([res.results[c]["out"] for c in range(N_CORES)], axis=0)



# revision 38
# speedup vs baseline: 1.0212x; 1.0206x over previous
"""Trainium2 Bass kernel for nn_Dictionary (vq_codebook): out = inp @ Q.T, Q from QR(weight+1e-8).

Strategy (per sharding_hint): data-parallel over batch B=131072 across 8 cores
(16384 rows each); Q.T replicated on every core (QR is tiny, computed on host).

Default mode "f16t2" (_build_t2): host transposes inp so the contraction dim i
lands on SBUF partitions with contiguous DMAs, operands in fp16 (1 cyc/row on
the PE, fp32 PSUM accumulation -> rel L2 err ~3.6e-4). Per core: stationary =
128x128 blocks of Q.T, moving = [128i, 512b] slices of inpT chunks, PSUM holds
out.T [128j, 512b] accumulated over 4 i-tiles (it-outer order so matmuls start
as each i-tile lands); DVE/ACT cast-copy PSUM into [128, <=4096] fp16 out.T
group tiles; host transposes back and upcasts to fp32.

The steady-state PE stream is at the fp16 floor (512 matmuls x 215.8 ns =
110.5us; median inter-MM gap = 216 ns), so v2 targets the edges (132.4 ->
~128.5us median):
  - 26 warmup matmuls on a DVE-memset zero tile keep the PE busy from ~7.5us
    (right after the fixed ~7us Tile/NEFF preamble) so the HAM clock gate
    un-throttles to 2.4 GHz around when real data lands (~10.2us)
  - qT rides the sync ring as 4 per-i-tile DMAs (lands ~9.3us); input chunks
    ride the scalar ring with a small leading chunk (512 cols)
  - input prefetch: 10 chunks, inpool bufs=5; output groups [3x4096, 2048,
    1024, 2x512] flush late-ish so early HBM bandwidth goes to input lead
  - last output group's 4 DMAs split across both HWDGE rings for the drain
Measured rejects (each ~1-4us slower and/or fragile — input starvation ->
HAM re-throttle outliers): fp8 (3.8e-2 L2 err > 2e-2 gate), 4096-wide input
chunks, single 3D-AP DMA per chunk (f16t3), dual-ring early input, outw=2048
groups, input-on-sync/output-on-scalar ring swap (f16t4), inpool bufs=6,
outpool bufs=3, input DMAs via GPSIMD/SWDGE from chunk 1 (f16t5, median
130.5us: kills the mid-run stall — steady-state DMA issue is no longer paced
by the ACT copy cadence — but SWDGE's ~1us-higher first-byte latency adds
ramp stalls) or from chunk 3 (f16t6, 142us: SWDGE ring warm-up lands in the
ramp-critical window, HAM oscillates).  Input DGE issue IS paced ~1/bn-iter
by in-order interleaving with ACT copies, but that still holds a ~1-chunk
lead and HWDGE transfer latency wins overall.
"""

import os

import numpy as np

import concourse.bacc as bacc
import concourse.mybir as mybir
import concourse.tile as tile
from concourse.bass_utils import run_bass_kernel_spmd

N_CORES = 8
B = 131072
D = 512  # contraction dim i (NUM_BASIS)
J = 512  # output dim j (MOTION_DIM)
BC = B // N_CORES  # rows per core
P = 128
KT = D // P  # 4 i-tiles

MODE = os.environ.get("KERNEL_MODE", "f16t2")  # f16t2 | f16t | f16 | bf16 | f32r | f16x3

_DT_IN = {
    "f16": mybir.dt.float16,
    "f16t": mybir.dt.float16,
    "f16t2": mybir.dt.float16,
    "bf16": mybir.dt.bfloat16,
    "f32r": mybir.dt.float32r,
    "f16x3": mybir.dt.float16,
}

_compiled = {}
LAST_RESULTS = None  # BassKernelResults of the most recent run (for test.py)


def _np_in_dtype(mode):
    if mode in ("f16", "f16t", "f16t2", "f16t3", "f16t4", "f16t5", "f16t6", "f16x3"):
        return np.float16
    if mode == "bf16":
        import ml_dtypes

        return ml_dtypes.bfloat16
    return np.float32


def _build(mode, bc=BC, chunk=4096, ob=4):
    dt_in = _DT_IN[mode]
    hilo = mode.endswith("x3")
    nc = bacc.Bacc()
    if hilo:
        inpT_hi = nc.dram_tensor("inpT_hi", [D, bc], dt_in, kind="ExternalInput")
        inpT_lo = nc.dram_tensor("inpT_lo", [D, bc], dt_in, kind="ExternalInput")
        qT_hi = nc.dram_tensor("qT_hi", [D, J], dt_in, kind="ExternalInput")
        qT_lo = nc.dram_tensor("qT_lo", [D, J], dt_in, kind="ExternalInput")
        in_drams = [inpT_hi, inpT_lo]
        q_drams = [qT_hi, qT_lo]
    else:
        inpT = nc.dram_tensor("inpT", [D, bc], dt_in, kind="ExternalInput")
        qT = nc.dram_tensor("qT", [D, J], dt_in, kind="ExternalInput")
        in_drams = [inpT]
        q_drams = [qT]
    out = nc.dram_tensor("out", [bc, J], mybir.dt.float32, kind="ExternalOutput")

    BCk = bc
    CHUNK = chunk  # b-columns fetched per supertile DMA (1 MB in fp16)
    OB = ob  # b-tiles batched per output DMA instruction

    # Output viewed as [p, ob-groups, j] so one DMA stores OB b-tiles.
    out3 = out.rearrange("(g ob p) j -> g p ob j", p=P, ob=OB)

    with tile.TileContext(nc) as tc:
        with (
            tc.tile_pool(name="qpool", bufs=1) as qpool,
            tc.tile_pool(name="inpool", bufs=2) as inpool,
            tc.tile_pool(name="outpool", bufs=3) as outpool,
            tc.tile_pool(name="psum", bufs=7, space="PSUM") as psum_pool,
        ):
            # Q.T tiles [i=128, j=512], static for the whole kernel.
            qts = []
            for qi, qd in enumerate(q_drams):
                for it in range(KT):
                    qt_t = qpool.tile([P, J], dt_in, tag=f"qt{qi}_{it}")
                    nc.sync.dma_start(out=qt_t[:], in_=qd[it * P : (it + 1) * P, :])
                    qts.append(qt_t)

            ot = None
            for chunk in range(BCk // CHUNK):
                csl = slice(chunk * CHUNK, (chunk + 1) * CHUNK)
                sups = []  # supertiles per (input, i-tile)
                for ii, ind in enumerate(in_drams):
                    for it in range(KT):
                        sup = inpool.tile([P, CHUNK], dt_in, tag=f"sup{ii}_{it}")
                        # input loads ride the ACT HWDGE ring; output the SP ring
                        nc.scalar.dma_start(
                            out=sup[:], in_=ind[it * P : (it + 1) * P, csl]
                        )
                        sups.append(sup)
                for bt in range(CHUNK // P):
                    bsl = slice(bt * P, (bt + 1) * P)
                    ps = psum_pool.tile([P, J], mybir.dt.float32, tag="ps")
                    if hilo:
                        # out = hi@Qhi + hi@Qlo + lo@Qhi  (drop lo@Qlo)
                        passes = [(0, 0), (0, 1), (1, 0)]
                    else:
                        passes = [(0, 0)]
                    n_mm = len(passes) * KT
                    mm = 0
                    for ii, qi in passes:
                        for it in range(KT):
                            nc.tensor.matmul(
                                ps[:],
                                sups[ii * KT + it][:, bsl],
                                qts[qi * KT + it][:],
                                start=(mm == 0),
                                stop=(mm == n_mm - 1),
                            )
                            mm += 1
                    gbt = chunk * (CHUNK // P) + bt  # global b-tile index
                    if gbt % OB == 0:
                        ot = outpool.tile([P, OB, J], mybir.dt.float32, tag="ot")
                    # split PSUM->SBUF copies across DVE and ACT
                    if gbt % 2 == 0:
                        nc.vector.tensor_copy(out=ot[:, gbt % OB, :], in_=ps[:])
                    else:
                        nc.scalar.copy(out=ot[:, gbt % OB, :], in_=ps[:])
                    if gbt % OB == OB - 1:
                        nc.sync.dma_start(out=out3[gbt // OB], in_=ot[:])
    nc.compile()
    return nc


def _build_t(mode, bc=BC, chunk=2048, outw=4096, warmup_mms=0):
    """Transposed-output variant: PSUM holds [j, b] tiles (stationary = Q.T
    128x128 blocks, moving = inpT [i, b] slices), output written as
    outT [J, bc] fp16 with wide per-partition runs, host transposes back.
    Halves output HBM traffic and keeps DMA packets large (>= 4 KB)."""
    dt_in = _DT_IN[mode]
    assert dt_in == mybir.dt.float16
    nc = bacc.Bacc()
    inpT = nc.dram_tensor("inpT", [D, bc], dt_in, kind="ExternalInput")
    qT = nc.dram_tensor("qT", [D, J], dt_in, kind="ExternalInput")
    outT = nc.dram_tensor("outT", [J, bc], mybir.dt.float16, kind="ExternalOutput")

    NB = 512  # moving free dim per matmul (one PSUM bank of fp32)
    JT = J // P  # 4 j-tiles

    # Input chunk schedule: uniform chunks (leading small chunk measured worse).
    plan = []
    rem = bc
    while rem > 0:
        c = min(chunk, rem)
        plan.append(c)
        rem -= c

    # Output group schedule: small groups at both ends (early first store,
    # short final flush), wide in the middle for large DMA packets.
    ow_plan = []
    rem = bc
    if bc >= 4 * outw:
        for c in (1024, 1024, 2048):
            ow_plan.append(c)
            rem -= c
    tail = [1024, 1024, 2048] if bc >= 4 * outw else []
    rem -= sum(tail)
    while rem > 0:
        c = min(outw, rem)
        ow_plan.append(c)
        rem -= c
    ow_plan.extend(reversed(tail))
    assert sum(ow_plan) == bc and all(w % 512 == 0 for w in ow_plan)
    # column index -> (group_idx, offset, width)
    col2grp = {}
    base = 0
    for gi, w in enumerate(ow_plan):
        for off in range(0, w, 512):
            col2grp[base + off] = (gi, off, w)
        base += w
    grp_base = {}
    base = 0
    for gi, w in enumerate(ow_plan):
        grp_base[gi] = base
        base += w

    with tile.TileContext(nc) as tc:
        with (
            tc.tile_pool(name="qpool", bufs=1) as qpool,
            tc.tile_pool(name="inpool", bufs=3) as inpool,
            tc.tile_pool(name="outpool", bufs=2) as outpool,
            tc.tile_pool(name="psum", bufs=8, space="PSUM") as psum_pool,
            tc.tile_pool(name="warm", bufs=1) as warm_pool,
            tc.tile_pool(name="warmps", bufs=1, space="PSUM") as warmps_pool,
        ):
            # Q.T rows for i-tile `it`: [128i, 512j]; stationary blocks are
            # 128-column slices qts[it][:, jt*128:(jt+1)*128]. (Dedicated
            # contiguous [128,128] weight tiles measured WORSE: 137.2us.)
            qts = []
            for it in range(KT):
                qt_t = qpool.tile([P, J], dt_in, tag=f"qt{it}")
                nc.gpsimd.dma_start(out=qt_t[:], in_=qT[it * P : (it + 1) * P, :])
                qts.append(qt_t)

            if warmup_mms:
                # Warmup matmuls on the (tiny, early-arriving) qT tiles: keeps
                # the PE HAM busy while the first input chunk streams in, so
                # real matmuls start un-throttled. Result bank is never read.
                wps = warmps_pool.tile([P, NB], mybir.dt.float32, tag="wps")
                for wi in range(warmup_mms):
                    nc.tensor.matmul(
                        wps[:],
                        qts[0][:, :P],
                        qts[0][:],
                        start=(wi == 0),
                        stop=(wi == warmup_mms - 1),
                    )

            ots = [None] * JT
            col_base = 0
            for chunk_i, csz in enumerate(plan):
                csl = slice(col_base, col_base + csz)
                sups = []
                for it in range(KT):
                    sup = inpool.tile([P, csz], dt_in, tag=f"sup{it}")
                    nc.scalar.dma_start(
                        out=sup[:], in_=inpT[it * P : (it + 1) * P, csl]
                    )
                    sups.append(sup)
                for bn in range(csz // NB):
                    col0 = col_base + bn * NB
                    gi, goff, gw = col2grp[col0]
                    if goff == 0:
                        for jt in range(JT):
                            ots[jt] = outpool.tile(
                                [P, outw],
                                mybir.dt.float16,
                                tag=f"ot{jt}",
                                name=f"ot{jt}",
                            )
                    osl = slice(goff, goff + NB)
                    bsl = slice(bn * NB, (bn + 1) * NB)
                    for jt in range(JT):
                        ps = psum_pool.tile([P, NB], mybir.dt.float32, tag="ps")
                        for it in range(KT):
                            nc.tensor.matmul(
                                ps[:],
                                qts[it][:, jt * P : (jt + 1) * P],
                                sups[it][:, bsl],
                                start=(it == 0),
                                stop=(it == KT - 1),
                            )
                        if jt % 2 == 0:
                            nc.vector.tensor_copy(out=ots[jt][:, osl], in_=ps[:])
                        else:
                            nc.scalar.copy(out=ots[jt][:, osl], in_=ps[:])
                    if goff + NB == gw:
                        g0 = grp_base[gi]
                        for jt in range(JT):
                            nc.sync.dma_start(
                                out=outT[jt * P : (jt + 1) * P, g0 : g0 + gw],
                                in_=ots[jt][:, :gw],
                            )
                col_base += csz
    nc.compile()
    return nc


def _build_t2(
    mode="f16t2",
    bc=BC,
    warmup=int(os.environ.get("KW", "26")),
    in_plan=None,
    ow_plan=None,
    inbufs=int(os.environ.get("KIB", "5")),
    outbufs=int(os.environ.get("KOB", "2")),
    swap_rings=False,
    swdge_in=False,
):
    """v2 of the transposed-output kernel. Changes vs _build_t (all aimed at
    ramp-in/ramp-out; steady state was already at the 216 ns/MM PE floor):
      - warmup matmuls on a DVE-memset zero tile, issued before any DMA lands,
        so the PE HAM un-throttles (K=8/8) before real matmuls start
      - Q.T loaded with ONE sync-ring (HWDGE) DMA instead of 4 SWDGE DMAs
        (landed at 10.4us before; sync ring is idle during ramp)
      - small leading input chunks so the first real matmul starts ~4us earlier
      - it-outer matmul order: matmuls for i-tile `it` only need sup[it], so
        compute starts as soon as the first quarter-chunk lands
      - outpool bufs=3 (group buffer recycle caused a 1.6us mid-stream stall)
      - output group plan ends small (512) to shorten the drain tail
    """
    dt_in = mybir.dt.float16
    nc = bacc.Bacc()
    inpT = nc.dram_tensor("inpT", [D, bc], dt_in, kind="ExternalInput")
    qT = nc.dram_tensor("qT", [D, J], dt_in, kind="ExternalInput")
    outT = nc.dram_tensor("outT", [J, bc], mybir.dt.float16, kind="ExternalOutput")

    NB = 512  # moving free dim per matmul (one PSUM bank of fp32)
    JT = J // P  # 4 j-tiles
    INW = 2048  # inpool tile width (chunks are subviews)

    if in_plan is None:
        in_plan = [512, 512, 1024] + [2048] * 7
    assert sum(in_plan) == bc and all(c % NB == 0 and c <= INW for c in in_plan)
    if ow_plan is None:
        ow_plan = [4096, 4096, 4096, 2048, 1024, 512, 512]
    assert sum(ow_plan) == bc and all(w % NB == 0 for w in ow_plan)
    outw = max(ow_plan)
    n_grp = len(ow_plan)

    # column index -> (group_idx, offset, width)
    col2grp = {}
    grp_base = {}
    base = 0
    for gi, w in enumerate(ow_plan):
        grp_base[gi] = base
        for off in range(0, w, NB):
            col2grp[base + off] = (gi, off, w)
        base += w

    # Q.T as [128, k, j]: one DMA for all 4 i-tiles
    qT2 = qT.rearrange("(k p) j -> p k j", p=P)

    with tile.TileContext(nc) as tc:
        with (
            tc.tile_pool(name="warm", bufs=1) as warm_pool,
            tc.tile_pool(name="qpool", bufs=1) as qpool,
            tc.tile_pool(name="inpool", bufs=inbufs) as inpool,
            tc.tile_pool(name="outpool", bufs=outbufs) as outpool,
            tc.tile_pool(name="psum", bufs=8, space="PSUM") as psum_pool,
        ):
            # --- PE warmup: matmuls on a zeroed tile, no DMA dependency ---
            if warmup:
                wt = warm_pool.tile([P, P], dt_in, tag="wt")
                nc.vector.memset(wt[:], 0.0)
                wps = psum_pool.tile([P, NB], mybir.dt.float32, tag="ps")
                for wi in range(warmup):
                    nc.tensor.matmul(
                        wps[:, :P],
                        wt[:],
                        wt[:],
                        start=(wi == 0),
                        stop=(wi == warmup - 1),
                    )

            # swap_rings: input DMAs alone on the sync(SP) queue so their
            # issue is gated only by inpool buffer recycle, not by the ACT
            # copy cadence (observed: input DGE on ACT paced 1 DMA per
            # bn-iter -> prefetch never ran ahead -> ~1.5us stall mid-run);
            # qt + output DMAs then ride the scalar(ACT) queue.
            q_eng = nc.scalar if swap_rings else nc.sync
            in_eng = nc.sync if swap_rings else nc.scalar
            out_eng = nc.scalar if swap_rings else nc.sync
            alt_eng = nc.sync if swap_rings else nc.scalar

            # --- Q.T: per-i-tile DMAs so the first matmul only
            # waits for the first 128 KB, not the full 512 KB ---
            qt = qpool.tile([P, KT, J], dt_in, tag="qt")
            for it in range(KT):
                q_eng.dma_start(out=qt[:, it, :], in_=qT2[:, it, :])

            def qw(it, jt):  # stationary [128i, 128j] block
                return qt[:, it, jt * P : (jt + 1) * P]

            ots = [None] * JT
            col_base = 0
            for chunk_i, csz in enumerate(in_plan):
                csl = slice(col_base, col_base + csz)
                # NOTE: routing early chunks over BOTH rings (dual-ring
                # input) measured ~2us WORSE across 6 runs.
                # swdge_in: steady-state chunks issue from the (otherwise
                # empty) GPSIMD queue, so DMA *issue* is gated only by the
                # inpool buffer semaphore — on ACT, in-order execution made
                # chunk N's DMAs wait behind chunk N-1's PSUM copies, so
                # the prefetch lead could never build. Chunk 0 stays on
                # HWDGE for its lower first-byte latency in the ramp.
                if swdge_in and chunk_i >= int(swdge_in):
                    ieng = nc.gpsimd
                else:
                    ieng = in_eng
                sups = []
                for it in range(KT):
                    sup = inpool.tile([P, INW], dt_in, tag=f"sup{it}")
                    ieng.dma_start(
                        out=sup[:, :csz], in_=inpT[it * P : (it + 1) * P, csl]
                    )
                    sups.append(sup)
                for bn in range(csz // NB):
                    col0 = col_base + bn * NB
                    gi, goff, gw = col2grp[col0]
                    if goff == 0:
                        for jt in range(JT):
                            ots[jt] = outpool.tile(
                                [P, outw],
                                mybir.dt.float16,
                                tag=f"ot{jt}",
                                name=f"ot{jt}",
                            )
                    osl = slice(goff, goff + NB)
                    bsl = slice(bn * NB, (bn + 1) * NB)
                    pss = [
                        psum_pool.tile([P, NB], mybir.dt.float32, tag="ps", name=f"ps{jt}")
                        for jt in range(JT)
                    ]
                    # it-outer: matmuls for i-tile `it` need only sups[it]
                    for it in range(KT):
                        for jt in range(JT):
                            nc.tensor.matmul(
                                pss[jt][:],
                                qw(it, jt),
                                sups[it][:, bsl],
                                start=(it == 0),
                                stop=(it == KT - 1),
                            )
                    for jt in range(JT):
                        if jt % 2 == 0:
                            nc.vector.tensor_copy(out=ots[jt][:, osl], in_=pss[jt][:])
                        else:
                            nc.scalar.copy(out=ots[jt][:, osl], in_=pss[jt][:])
                    if goff + NB == gw:
                        g0 = grp_base[gi]
                        for jt in range(JT):
                            # final group: input ring is idle, split across both
                            eng = (
                                alt_eng
                                if (gi == n_grp - 1 and jt % 2 == 1)
                                else out_eng
                            )
                            eng.dma_start(
                                out=outT[jt * P : (jt + 1) * P, g0 : g0 + gw],
                                in_=ots[jt][:, :gw],
                            )
                col_base += csz
    nc.compile()
    return nc


def _build_t3(
    mode="f16t3",
    bc=BC,
    warmup=int(os.environ.get("KW", "26")),
    in_plan=None,
    ow_plan=None,
    inbufs=int(os.environ.get("KIB", "5")),
    outbufs=int(os.environ.get("KOB", "2")),
):
    """v3 of the transposed-output kernel. vs _build_t2:
      - each input chunk is ONE 3D-AP DMA (all 4 i-tiles) on the sync ring:
        4x less HWDGE descriptor-gen time on the issuing engine, and input
        DGE no longer shares a queue with PSUM copies
      - qt + output groups ride the scalar(ACT) ring
      - PSUM copies rebalanced: jt0..2 on DVE, jt3 on ACT (ACT also runs
        the output DGE bursts)
    """
    dt_in = mybir.dt.float16
    nc = bacc.Bacc()
    inpT = nc.dram_tensor("inpT", [D, bc], dt_in, kind="ExternalInput")
    qT = nc.dram_tensor("qT", [D, J], dt_in, kind="ExternalInput")
    outT = nc.dram_tensor("outT", [J, bc], mybir.dt.float16, kind="ExternalOutput")

    NB = 512
    JT = J // P
    INW = 2048

    if in_plan is None:
        in_plan = [512, 512, 1024] + [2048] * 7
    assert sum(in_plan) == bc and all(c % NB == 0 and c <= INW for c in in_plan)
    if ow_plan is None:
        ow_plan = [4096, 4096, 4096, 2048, 1024, 512, 512]
    assert sum(ow_plan) == bc and all(w % NB == 0 for w in ow_plan)
    outw = max(ow_plan)
    n_grp = len(ow_plan)

    col2grp = {}
    grp_base = {}
    base = 0
    for gi, w in enumerate(ow_plan):
        grp_base[gi] = base
        for off in range(0, w, NB):
            col2grp[base + off] = (gi, off, w)
        base += w

    qT2 = qT.rearrange("(k p) j -> p k j", p=P)
    inpT3 = inpT.rearrange("(k p) b -> p k b", p=P)

    with tile.TileContext(nc) as tc:
        with (
            tc.tile_pool(name="warm", bufs=1) as warm_pool,
            tc.tile_pool(name="qpool", bufs=1) as qpool,
            tc.tile_pool(name="inpool", bufs=inbufs) as inpool,
            tc.tile_pool(name="outpool", bufs=outbufs) as outpool,
            tc.tile_pool(name="psum", bufs=8, space="PSUM") as psum_pool,
        ):
            if warmup:
                wt = warm_pool.tile([P, P], dt_in, tag="wt")
                nc.vector.memset(wt[:], 0.0)
                wps = psum_pool.tile([P, NB], mybir.dt.float32, tag="ps")
                for wi in range(warmup):
                    nc.tensor.matmul(
                        wps[:, :P],
                        wt[:],
                        wt[:],
                        start=(wi == 0),
                        stop=(wi == warmup - 1),
                    )

            qt = qpool.tile([P, KT, J], dt_in, tag="qt")
            for it in range(KT):
                nc.scalar.dma_start(out=qt[:, it, :], in_=qT2[:, it, :])

            def qw(it, jt):
                return qt[:, it, jt * P : (jt + 1) * P]

            ots = [None] * JT
            col_base = 0
            for chunk_i, csz in enumerate(in_plan):
                csl = slice(col_base, col_base + csz)
                sup = inpool.tile([P, KT, INW], dt_in, tag="sup")
                nc.sync.dma_start(out=sup[:, :, :csz], in_=inpT3[:, :, csl])
                for bn in range(csz // NB):
                    col0 = col_base + bn * NB
                    gi, goff, gw = col2grp[col0]
                    if goff == 0:
                        for jt in range(JT):
                            ots[jt] = outpool.tile(
                                [P, outw],
                                mybir.dt.float16,
                                tag=f"ot{jt}",
                                name=f"ot{jt}",
                            )
                    osl = slice(goff, goff + NB)
                    bsl = slice(bn * NB, (bn + 1) * NB)
                    pss = [
                        psum_pool.tile([P, NB], mybir.dt.float32, tag="ps", name=f"ps{jt}")
                        for jt in range(JT)
                    ]
                    for it in range(KT):
                        for jt in range(JT):
                            nc.tensor.matmul(
                                pss[jt][:],
                                qw(it, jt),
                                sup[:, it, bsl],
                                start=(it == 0),
                                stop=(it == KT - 1),
                            )
                    for jt in range(JT):
                        if jt < 3:
                            nc.vector.tensor_copy(out=ots[jt][:, osl], in_=pss[jt][:])
                        else:
                            nc.scalar.copy(out=ots[jt][:, osl], in_=pss[jt][:])
                    if goff + NB == gw:
                        g0 = grp_base[gi]
                        for jt in range(JT):
                            eng = (
                                nc.sync
                                if (gi == n_grp - 1 and jt % 2 == 1)
                                else nc.scalar
                            )
                            eng.dma_start(
                                out=outT[jt * P : (jt + 1) * P, g0 : g0 + gw],
                                in_=ots[jt][:, :gw],
                            )
                col_base += csz
    nc.compile()
    return nc


def _get_nc(mode):
    if mode not in _compiled:
        if mode == "f16t6":
            _compiled[mode] = _build_t2(mode, swdge_in=3)
        elif mode == "f16t5":
            _compiled[mode] = _build_t2(mode, swdge_in=1)
        elif mode == "f16t4":
            _compiled[mode] = _build_t2(mode, swap_rings=True)
        elif mode == "f16t3":
            _compiled[mode] = _build_t3(mode)
        elif mode == "f16t2":
            _compiled[mode] = _build_t2(mode)
        elif mode == "f16t":
            _compiled[mode] = _build_t(mode)
        else:
            _compiled[mode] = _build(mode)
    return _compiled[mode]


def kernel(inp: np.ndarray, weight: np.ndarray) -> np.ndarray:
    global LAST_RESULTS
    mode = MODE
    nc = _get_nc(mode)

    w = np.asarray(weight, dtype=np.float32) + np.float32(1e-8)
    Q = np.linalg.qr(w)[0].astype(np.float32)  # [J, D] == [512, 512]
    np_dt = _np_in_dtype(mode)

    inp = np.asarray(inp, dtype=np.float32)
    inpT = inp.T  # [D, B] view

    QT = Q.T  # QT[i, j] = Q[j, i]
    in_maps = []
    if mode.endswith("x3"):
        qt_hi = QT.astype(np_dt)
        qt_lo = (QT - qt_hi.astype(np.float32)).astype(np_dt)
        for c in range(N_CORES):
            sl = inpT[:, c * BC : (c + 1) * BC].astype(np.float32)
            hi = sl.astype(np_dt)
            lo = (sl - hi.astype(np.float32)).astype(np_dt)
            in_maps.append(
                {"inpT_hi": hi, "inpT_lo": lo, "qT_hi": qt_hi, "qT_lo": qt_lo}
            )
    else:
        qt16 = np.ascontiguousarray(QT).astype(np_dt)
        for c in range(N_CORES):
            in_maps.append(
                {"inpT": inpT[:, c * BC : (c + 1) * BC].astype(np_dt), "qT": qt16}
            )

    # First execution of a freshly compiled NEFF occasionally dies with
    # NRT_EXEC_UNIT_UNRECOVERABLE (transient, esp. with profiling on);
    # a straight retry has always succeeded.
    last_exc = None
    for _attempt in range(3):
        try:
            res = run_bass_kernel_spmd(nc, in_maps, list(range(N_CORES)))
            break
        except Exception as e:  # noqa: BLE001
            last_exc = e
            import time as _time

            _time.sleep(2.0)
    else:
        raise last_exc
    LAST_RESULTS = res
    if mode in ("f16t", "f16t2", "f16t3", "f16t4", "f16t5", "f16t6"):
        out = np.empty((B, J), dtype=np.float32)
        for c in range(N_CORES):
            # outT [J, BC] fp16 -> out rows [c*BC:(c+1)*BC] fp32
            out[c * BC : (c + 1) * BC, :] = res.results[c]["outT"].T
        return out
    return np.concatenate([res.results[c]["out"] for c in range(N_CORES)], axis=0)

